# revision 11
# baseline (speedup 1.0000x reference)
import os
import sys
import types

import numpy as np

sys.path.insert(0, "/opt/trn_rl_repo")

import ml_dtypes  # noqa: E402
import concourse.mybir as mybir  # noqa: E402
import concourse.tile as tile  # noqa: E402
from concourse import bacc  # noqa: E402
from concourse.bass import ds, ts  # noqa: E402
from concourse.bass_utils import run_bass_kernel_spmd  # noqa: E402

BF16 = mybir.dt.bfloat16
F32 = mybir.dt.float32
FP8 = mybir.dt.float8e4
I16 = mybir.dt.int16
bfdt = ml_dtypes.bfloat16
AF = mybir.ActivationFunctionType
ALU = mybir.AluOpType
DR = mybir.MatmulPerfMode.DoubleRow

B, D, N = 4, 512, 2048
H, KVH, DH = 8, 2, 64
CONTEXT_LEN = 4096
NLOC = 1024  # tokens per core
P = 128
NCORES = 8
HP = H // 2  # head pairs
NCH = N // P  # 16 key chunks of 128
NCH_LOC = NLOC // P  # 8 local key chunks

# exp is computed as exp(0.125*s + EBIAS) so fp8e4m3 never overflows
# (max scaled score measured 8.29 -> max value ~123, NaN needs >464; fp8
# relative precision is scale-free so the extra headroom is cheap). The
# constant cancels between AV numerator and the ones-row denominator.
EBIAS = -3.4808042
# DVE Schraudolph exp: bf16 bits = floor(s*SCH_A + SCH_B) (DVE f32->i16
# convert truncates); C=6.0 calibrated for min max-rel-err (~3.7%).
SCH_A = 0.125 * 128.0 / float(np.log(2.0))
SCH_B = 16256.0 - 6.0 + EBIAS * 128.0 / float(np.log(2.0))

_CACHE = {}


def _enable_trace_hook():
    """Register the NTFF profile hook (missing antenv.axon_hooks shim)."""
    try:
        import antenv

        if "antenv.axon_hooks" in sys.modules:
            return
        mod = types.ModuleType("antenv.axon_hooks")

        def set_axon_ntff_profile_hook(h):
            mod._hook = h

        def get_axon_ntff_profile_hook():
            return getattr(mod, "_hook", None)

        mod.set_axon_ntff_profile_hook = set_axon_ntff_profile_hook
        mod.get_axon_ntff_profile_hook = get_axon_ntff_profile_hook
        sys.modules["antenv.axon_hooks"] = mod
        antenv.axon_hooks = mod
        from trn_agent_boot.trn_boot import _ntff_profile_via_ctypes

        set_axon_ntff_profile_hook(_ntff_profile_via_ctypes("/opt/axon/libaxon_pjrt.so"))
    except Exception:
        pass


def _build(TRIVIAL_GB, TRIVIAL_BO):
    nc = bacc.Bacc(None, target_bir_lowering=False, debug=False)
    dp = nc.declare_dram_parameter

    x_e = dp("x", [4, P, NLOC], F32, isOutput=False)
    wq_e = dp("wq", [P, 4, 512], BF16, isOutput=False)
    wqs_e = dp("wqs", [P, 4, 512], BF16, isOutput=False)
    wk_e = dp("wk", [P, 4, 128], BF16, isOutput=False)
    wks_e = dp("wks", [P, 4, 128], BF16, isOutput=False)
    wv_e = dp("wv", [P, 4, 128], BF16, isOutput=False)
    wo_e = dp("wo", [P, 4, 512], BF16, isOutput=False)
    cq_e = dp("cq", [P, NLOC], BF16, isOutput=False)
    sq_e = dp("sq", [P, NLOC], BF16, isOutput=False)
    ck_e = dp("ck", [P, NLOC], BF16, isOutput=False)
    sk_e = dp("sk", [P, NLOC], BF16, isOutput=False)
    gam_e = dp("gam", [P, 4], F32, isOutput=False)  # gamma per (p, chunk)
    bet_e = dp("bet", [P, 4], F32, isOutput=False)  # beta per (p, chunk)
    bo_e = dp("bo", [P, 4], F32, isOutput=False)  # bout per (p, chunk)
    ones_e = dp("ones", [P, 1], F32, isOutput=False)
    onesb_e = dp("onesb", [P, 1], BF16, isOutput=False)
    out_e = dp("out", [4, P, NLOC], F32, isOutput=True)

    with tile.TileContext(nc) as tc:
        with (
            tc.tile_pool(name="persist", bufs=1) as PS,
            tc.tile_pool(name="tmp", bufs=2) as TMP,
            tc.tile_pool(name="tmp4", bufs=4) as TMP4,
            tc.tile_pool(name="e8", bufs=2) as E8P,
            tc.tile_pool(name="e16", bufs=1) as E16P,
            tc.tile_pool(name="dram", bufs=1, space="DRAM") as DRAM,
        ):
            # ---------------- phase A: inputs -> SBUF ----------------
            SQP_cm = tc.tile_pool(name="sq_pool", bufs=1)
            SQP = SQP_cm.__enter__()
            x_sb = [
                [SQP.tile([P, 512], F32, name=f"x{c}_{tq}") for tq in range(2)]
                for c in range(4)
            ]
            for c in range(4):
                for tq in range(2):
                    nc.sync.dma_start(x_sb[c][tq][:], x_e[c][:, ts(tq, 512)])
            ones_sb = PS.tile([P, 1], F32, name="ones")
            nc.sync.dma_start(ones_sb[:], ones_e[:])
            onesb_sb = PS.tile([P, 1], BF16, name="onesb")
            nc.sync.dma_start(onesb_sb[:], onesb_e[:])
            ebias_sb = PS.tile([P, 1], F32, name="ebias")
            nc.gpsimd.memset(ebias_sb[:], EBIAS)
            gam_sb = PS.tile([P, 4], F32, name="gam")
            nc.sync.dma_start(gam_sb[:], gam_e[:])
            bet_sb = PS.tile([P, 4], F32, name="bet")
            nc.sync.dma_start(bet_sb[:], bet_e[:])
            bo_sb = PS.tile([P, 4], F32, name="bo")
            nc.sync.dma_start(bo_sb[:], bo_e[:])
            wk_sb = PS.tile([P, 4, 128], BF16, name="wk")
            nc.sync.dma_start(wk_sb[:], wk_e[:])
            wks_sb = PS.tile([P, 4, 128], BF16, name="wks")
            nc.sync.dma_start(wks_sb[:], wks_e[:])
            wv_sb = PS.tile([P, 4, 128], BF16, name="wv")
            nc.sync.dma_start(wv_sb[:], wv_e[:])
            ck_sb = PS.tile([P, NLOC], BF16, name="ck")
            nc.sync.dma_start(ck_sb[:], ck_e[:])
            sk_sb = PS.tile([P, NLOC], BF16, name="sk")
            nc.sync.dma_start(sk_sb[:], sk_e[:])
            wq_sb = PS.tile([P, 4, 512], BF16, name="wq")
            nc.sync.dma_start(wq_sb[:], wq_e[:])
            wqs_sb = PS.tile([P, 4, 512], BF16, name="wqs")
            nc.sync.dma_start(wqs_sb[:], wqs_e[:])
            cq_sb = PS.tile([P, NLOC], BF16, name="cq")
            nc.sync.dma_start(cq_sb[:], cq_e[:])
            sq_sb = PS.tile([P, NLOC], BF16, name="sq")
            nc.sync.dma_start(sq_sb[:], sq_e[:])
            wo_sb = PS.tile([P, 4, 512], BF16, name="wo")
            nc.sync.dma_start(wo_sb[:], wo_e[:])

            # v lhsT stores. bf16 copies (slot = 2*chunk + par) feed the
            # DVE-exp (Schraudolph) segments; fp8 copies (per-par, chunk
            # sequential) feed the DoubleRow segments. Col DH is the ones
            # column that accumulates the softmax denominator.
            v_loc = PS.tile([P, 2 * NCH_LOC, DH + 1], BF16, name="vloc")
            nc.gpsimd.memset(v_loc[:, :, DH : DH + 1], 1.0)
            v_rem = [
                PS.tile([P, NCH_LOC, DH + 1], BF16, name=f"vrem{h}") for h in range(2)
            ]
            for h in range(2):
                nc.gpsimd.memset(v_rem[h][:, :, DH : DH + 1], 1.0)
            # pitch 80 (16-byte aligned pair stride for DoubleRow LDWEIGHTS);
            # data in cols 0:65 (V | ones)
            v8_loc = [PS.tile([P, NCH_LOC, 80], FP8, name=f"v8l{g}") for g in range(2)]
            v8_rem = [PS.tile([P, NCH_LOC, 80], FP8, name=f"v8r{g}") for g in range(2)]
            for g in range(2):
                nc.gpsimd.memset(v8_loc[g][:, :, DH : DH + 1], 1.0)
                nc.gpsimd.memset(v8_rem[g][:, :, DH : DH + 1], 1.0)

            xnb = [PS.tile([P, NLOC], BF16, name=f"xnb{c}") for c in range(4)]
            qr_sb = [PS.tile([P, NLOC], BF16, name=f"qr{i}") for i in range(HP)]
            k_bf = PS.tile([P, NLOC], BF16, name="kbf")
            k_rem = [PS.tile([P, 512], BF16, name=f"krem{h}") for h in range(2)]
            vcp_sb = PS.tile([P, NCH_LOC, 128], BF16, name="vcp")
            ohat = [PS.tile([P, NLOC], BF16, name=f"oh{i}") for i in range(HP)]

            ag_in = DRAM.tile([2, P, NLOC], BF16)
            ag_out = DRAM.tile([2, 2, P, NLOC], BF16)

            # ---------------- phase B: layernorm ----------------
            with tc.tile_pool(name="ps_b1", bufs=1, space="PSUM") as PB1:
                stats = PB1.tile([1, 4, 512], F32, name="stats")
                xsq = [
                    [SQP.tile([P, 512], BF16, name=f"xsq{c}_{tq}") for tq in range(2)]
                    for c in range(4)
                ]
                for c in range(4):
                    for h2 in range(2):
                        nc.vector.tensor_mul(
                            xsq[c][h2][:], x_sb[c][h2][:], x_sb[c][h2][:]
                        )
                for tq in range(2):
                    for c in range(4):
                        nc.tensor.matmul(
                            stats[:, tq, :], ones_sb[:], x_sb[c][tq][:],
                            start=(c == 0), stop=(c == 3),
                        )
                for tq in range(2):
                    for c in range(4):
                        nc.tensor.matmul(
                            stats[:, 2 + tq, :], onesb_sb[:], xsq[c][tq][:],
                            start=(c == 0), stop=(c == 3),
                        )
                mu_sb = TMP4.tile([1, NLOC], F32, tag="ln")
                ex2_sb = TMP4.tile([1, NLOC], F32, tag="ln")
                musq_sb = TMP4.tile([1, NLOC], F32, tag="ln")
                var_sb = TMP4.tile([1, NLOC], F32, tag="ln")
                nc.scalar.mul(mu_sb[:], stats[:, 0:2, :].rearrange("p a b -> p (a b)"), 1.0 / 512)
                nc.scalar.activation(
                    ex2_sb[:], stats[:, 2:4, :].rearrange("p a b -> p (a b)"),
                    AF.Copy, bias=1e-5, scale=1.0 / 512,
                )
                nc.vector.tensor_mul(musq_sb[:], mu_sb[:], mu_sb[:])
                nc.vector.tensor_tensor(var_sb[:], ex2_sb[:], musq_sb[:], ALU.subtract)
                # rstd = exp(-0.5 * ln(var)) -- Ln and Exp share a table set
                sd_sb = TMP4.tile([1, NLOC], F32, tag="ln")
                rstd_sb = TMP4.tile([1, NLOC], F32, tag="ln")
                nc.scalar.activation(sd_sb[:], var_sb[:], AF.Ln)
                nc.scalar.activation(rstd_sb[:], sd_sb[:], AF.Exp, scale=-0.5)
                # rstd and mu*rstd broadcast to all 128 partitions
                rstd_bc = SQP.tile([P, NLOC], F32, name="rstdbc")
                nc.gpsimd.partition_broadcast(rstd_bc[:], rstd_sb[0:1, :])
                mrs_sb = TMP4.tile([1, NLOC], F32, tag="ln")
                nc.vector.tensor_mul(mrs_sb[:], mu_sb[:], rstd_sb[:])
                mrs_bc = SQP.tile([P, NLOC], F32, name="mrsbc")
                nc.gpsimd.partition_broadcast(mrs_bc[:], mrs_sb[0:1, :])

            # xn = ((x * rstd) - mu*rstd) [* gamma + beta]   (bf16 out)
            for c in range(4):
                for tq in range(2):
                    t1 = TMP.tile([P, 512], F32, tag="th")
                    nc.vector.tensor_mul(
                        t1[:], x_sb[c][tq][:], rstd_bc[:, ts(tq, 512)]
                    )
                    if TRIVIAL_GB:
                        nc.vector.tensor_tensor(
                            xnb[c][:, ts(tq, 512)], t1[:],
                            mrs_bc[:, ts(tq, 512)], ALU.subtract,
                        )
                    else:
                        t2 = TMP.tile([P, 512], F32, tag="th")
                        nc.vector.tensor_tensor(
                            t2[:], t1[:], mrs_bc[:, ts(tq, 512)], ALU.subtract
                        )
                        nc.vector.tensor_scalar(
                            xnb[c][:, ts(tq, 512)], t2[:],
                            gam_sb[:, c : c + 1], bet_sb[:, c : c + 1],
                            ALU.mult, ALU.add,
                        )
            SQP_cm.__exit__(None, None, None)

            # ---------------- phase C1: k/v projection, rotary, allgather ----------------
            with tc.tile_pool(name="ps_c1", bufs=1, space="PSUM") as PC:
                kp0 = PC.tile([P, 2, 512], F32, name="kp0")
                kp1 = PC.tile([P, 2, 512], F32, name="kp1")
                for sw, (kps, w) in enumerate(((kp0, wk_sb), (kp1, wks_sb))):
                    for tq in range(2):
                        for c in range(4):
                            nc.tensor.matmul(
                                kps[:, tq, :], w[:, c, :], xnb[c][:, ts(tq, 512)],
                                start=(c == 0), stop=(c == 3),
                            )
                t1 = TMP.tile([P, NLOC], F32, tag="t")
                t2 = TMP.tile([P, NLOC], F32, tag="t")
                nc.vector.tensor_mul(t1[:], ck_sb[:], kp0[:].rearrange("p a b -> p (a b)"))
                nc.vector.tensor_mul(t2[:], sk_sb[:], kp1[:].rearrange("p a b -> p (a b)"))
                nc.vector.tensor_add(k_bf[:], t1[:], t2[:])

                for c8 in range(NCH_LOC):
                    vp = PC.tile([P, 128], F32, name=f"vp{c8 % 2}")
                    for c in range(4):
                        nc.tensor.matmul(
                            vp[:], xnb[c][:, ts(c8, 128)], wv_sb[:, c, :],
                            start=(c == 0), stop=(c == 3),
                        )
                    nc.vector.tensor_copy(vcp_sb[:, c8, :], vp[:])

                nc.sync.dma_start(ag_in[0], k_bf[:])
                nc.sync.dma_start(
                    ag_in[1], vcp_sb[:].rearrange("p a b -> p (a b)")
                )
                nc.gpsimd.collective_compute(
                    "AllGather",
                    ALU.bypass,
                    ins=[ag_in[:]],
                    outs=[ag_out[:]],
                    replica_groups=[[0, 1], [2, 3], [4, 5], [6, 7]],
                )

            # local v -> bf16 slots 0..15 and fp8 per-par tiles
            nc.vector.tensor_copy(
                v_loc[:, :, 0:DH],
                vcp_sb[:].rearrange("p a (g d) -> p (a g) d", g=2),
            )
            for g in range(2):
                nc.vector.tensor_copy(
                    v8_loc[g][:, :, 0:DH], vcp_sb[:, :, ts(g, DH)]
                )

            # ---------------- phase D: attention main loop ----------------
            # Per stream segment (one (hp, tq) x 8-chunk half): scores are
            # matmul'd in groups of 3 slots (slot = 2*ci+par), exp'd to an
            # E tile (slot-sequential), and AV-accumulated. ACT segments
            # produce fp8 and use DoubleRow AV over adjacent chunk pairs;
            # DVE segments produce Schraudolph bf16 (int16 bitcast) and use
            # plain bf16 AV. Denominator rides in the V ones column.
            spills = {}

            def emit_qproj(PSC, i):
                qc = TMP4.tile([P, NLOC], BF16, tag="qcs")
                qs = TMP4.tile([P, NLOC], BF16, tag="qcs")
                for tq in range(2):
                    qps = PSC.tile([P, 3, 512], F32, tag="sc", name="qps")
                    for sl, w in ((0, wq_sb), (1, wqs_sb)):
                        for c in range(4):
                            nc.tensor.matmul(
                                qps[:, sl, :], w[:, c, ts(i, 128)],
                                xnb[c][:, ts(tq, 512)],
                                start=(c == 0), stop=(c == 3),
                            )
                    nc.scalar.copy(qc[:, ts(tq, 512)], qps[:, 0, :])
                    nc.scalar.copy(qs[:, ts(tq, 512)], qps[:, 1, :])
                t1 = TMP.tile([P, NLOC], BF16, tag="qt")
                t2 = TMP.tile([P, NLOC], BF16, tag="qt")
                nc.vector.tensor_mul(t1[:], cq_sb[:], qc[:])
                nc.vector.tensor_mul(t2[:], sq_sb[:], qs[:])
                nc.vector.tensor_add(qr_sb[i][:], t1[:], t2[:])

            def emit_epilogue(hp, tq, oA, oB, restore):
                sA = TMP.tile([DH + 1, 512], F32, tag="sum")
                sB = TMP.tile([DH + 1, 512], F32, tag="sum")
                if restore:
                    cpA, cpB = spills[(hp, tq)]
                    nc.vector.tensor_add(sA[:], oA[:], cpA[:])
                    nc.vector.tensor_add(sB[:], oB[:], cpB[:])
                else:
                    nc.vector.tensor_copy(sA[:], oA[:])
                    nc.vector.tensor_copy(sB[:], oB[:])
                den2 = TMP.tile([1, 1024], F32, tag="den")
                nc.vector.tensor_copy(den2[0:1, 0:512], sA[DH : DH + 1, :])
                nc.vector.tensor_copy(den2[0:1, 512:1024], sB[DH : DH + 1, :])
                rec2 = TMP.tile([1, 1024], F32, tag="den")
                nc.vector.reciprocal_approx_fast(rec2[:], den2[:])
                pbA = TMP.tile([64, 512], F32, tag="pb")
                pbB = TMP.tile([64, 512], F32, tag="pb")
                nc.gpsimd.partition_broadcast(pbA[:], rec2[0:1, 0:512])
                nc.gpsimd.partition_broadcast(pbB[:], rec2[0:1, 512:1024])
                nc.vector.tensor_mul(ohat[hp][0:64, ts(tq, 512)], sA[0:DH, :], pbA[:])
                nc.vector.tensor_mul(ohat[hp][64:128, ts(tq, 512)], sB[0:DH, :], pbB[:])

            def emit_spill(hp, tq, oA, oB):
                cpA = PS.tile([DH + 1, 512], F32, name=f"spA{hp}{tq}")
                cpB = PS.tile([DH + 1, 512], F32, name=f"spB{hp}{tq}")
                nc.vector.tensor_copy(cpA[:], oA[:])
                nc.vector.tensor_copy(cpB[:], oB[:])
                spills[(hp, tq)] = (cpA, cpB)

            def run_stream(PSC, PAV, plan, hook=None):
                """plan: list of (hp, tq, chunks, mode, eng).

                chunks is a list of 8 or 16 chunk ids; eng is 'act' (fp8 +
                DoubleRow AV) or 'dve' (Schraudolph bf16 AV). Pipeline per
                3-slot group: scores | AV of ready pairs | exp.
                """
                groups = []  # (seg, half_key, hch, [slot descriptors])
                etiles = {}  # half_key -> E tile, allocated lazily at first use
                for hp, tq, chunks, mode, eng in plan:
                    nunits = 2 * len(chunks)
                    seg = {
                        "hp": hp, "tq": tq, "mode": mode, "eng": eng,
                        "nunits": nunits, "done_units": {0: 0, 1: 0},
                        "oA": None, "oB": None,
                    }
                    halves = [chunks[i : i + 8] for i in range(0, len(chunks), 8)]
                    for hi, hch in enumerate(halves):
                        hkey = (hp, tq, mode, hi)
                        slots = []
                        for ci, ch in enumerate(hch):
                            for par in range(2):
                                slots.append((par, ci, ch))
                        for gs in range(0, 16, 3):
                            groups.append((seg, hkey, hch, slots[gs : gs + 3]))

                def get_etile(seg, hkey):
                    if hkey not in etiles:
                        if seg["eng"] == "act":
                            etiles[hkey] = E8P.tile([P, 16, 512], FP8, tag="e8", name="et8")
                        else:
                            etiles[hkey] = E16P.tile([P, 16, 512], I16, tag="e16", name="et16")
                    return etiles[hkey]

                def emit_av_unit(seg, par, ch, rhs, first, last, dr):
                    if first:
                        if par == 0:
                            seg["oA"] = PAV.tile([DH + 1, 512], F32, tag="avA", name="av_a")
                        else:
                            seg["oB"] = PAV.tile([DH + 1, 512], F32, tag="avB", name="av_b")
                    o = seg["oA"] if par == 0 else seg["oB"]
                    if dr:
                        if ch < NCH_LOC:
                            vt = v8_loc[par][:, ch : ch + 2, 0 : DH + 1]
                        else:
                            vt = v8_rem[par][:, ch - NCH_LOC : ch - NCH_LOC + 2, 0 : DH + 1]
                        nc.tensor.matmul(
                            o[:], vt, rhs, start=first, stop=last, perf_mode=DR
                        )
                        seg["done_units"][par] += 2
                    else:
                        if ch < NCH_LOC:
                            vt = v_loc[:, 2 * ch + par, :]
                        else:
                            cr = ch - NCH_LOC
                            vt = v_rem[cr // 4][:, 2 * (cr % 4) + par, :]
                        nc.tensor.matmul(o[:], vt, rhs, start=first, stop=last)
                        seg["done_units"][par] += 1
                    if (
                        seg["done_units"][0] + seg["done_units"][1]
                        == seg["nunits"]
                    ):
                        oA, oB = seg["oA"], seg["oB"]
                        if seg["mode"] == "spill":
                            emit_spill(seg["hp"], seg["tq"], oA, oB)
                        else:
                            emit_epilogue(
                                seg["hp"], seg["tq"], oA, oB,
                                seg["mode"] == "epi_restore",
                            )

                pending = []  # (trigger_gidx, emit_fn)
                hook_at = len(groups) // 2
                for gidx, item in enumerate(groups + [None] * 2):
                    seg, hkey, hch, slots = item if item is not None else (None,) * 4
                    et = get_etile(seg, hkey) if seg is not None else None
                    if gidx == hook_at and hook is not None:
                        hook()
                    if seg is not None:
                        # scores for this group
                        sc = PSC.tile([P, 3, 512], F32, tag="sc")
                        hp, tq = seg["hp"], seg["tq"]
                        for pos, (par, ci, ch) in enumerate(slots):
                            if ch < NCH_LOC:
                                ksrc = k_bf[:, ts(ch, 128)]
                            else:
                                cr = ch - NCH_LOC
                                ksrc = k_rem[cr // 4][:, ts(cr % 4, 128)]
                            nc.tensor.matmul(
                                sc[:, pos, :],
                                ksrc[64 * par : 64 * (par + 1), :],
                                qr_sb[hp][64 * par : 64 * (par + 1), ts(tq, 512)],
                                start=True, stop=True,
                                tile_position=(64 * par, 0),
                            )
                    # AV matmuls whose exps are already emitted
                    while pending and pending[0][0] < gidx:
                        pending.pop(0)[1]()
                    if seg is not None:
                        # exp for this group on the segment's engine
                        ns = len(slots)
                        s0 = 2 * slots[0][1] + slots[0][0]
                        if seg["eng"] == "act":
                            nc.scalar.activation(
                                et[:, s0 : s0 + ns, :].rearrange("p a b -> p (a b)"),
                                sc[:, 0:ns, :].rearrange("p a b -> p (a b)"),
                                AF.Exp, bias=ebias_sb[:], scale=0.125,
                            )
                        else:
                            nc.vector.tensor_scalar(
                                et[:, s0 : s0 + ns, :].rearrange("p a b -> p (a b)"),
                                sc[:, 0:ns, :].rearrange("p a b -> p (a b)"),
                                SCH_A, SCH_B, ALU.mult, ALU.add,
                            )
                        # register completed AV units
                        for par, ci, ch in slots:
                            seg_, et_, hch_ = seg, et, hch
                            if seg["eng"] == "act":
                                # pair (even ci, odd ci): fires on odd ci slot
                                if ci % 2 == 1:
                                    j = ci // 2
                                    pch = hch_[2 * j]
                                    ev = et_[:].rearrange(
                                        "p (j c2 p2) n -> p j c2 p2 n", j=4, c2=2, p2=2
                                    )
                                    rhs = ev[:, j, :, par, :]
                                    nu = seg_["sched_units"] = seg_.get("sched_units", {0: 0, 1: 0})
                                    nu[par] += 2
                                    first_u = nu[par] == 2
                                    last_u = nu[par] == seg_["nunits"] // 2
                                    pending.append(
                                        (
                                            gidx,
                                            (lambda s=seg_, p=par, c=pch, r=rhs,
                                             f=first_u, l=last_u: emit_av_unit(
                                                s, p, c, r, f, l, True
                                            )),
                                        )
                                    )
                            else:
                                j = 2 * ci + par
                                rhs = et_[:].bitcast(BF16)[:, j, :]
                                nu = seg_["sched_units"] = seg_.get("sched_units", {0: 0, 1: 0})
                                nu[par] += 1
                                first_u = nu[par] == 1
                                last_u = nu[par] == seg_["nunits"] // 2
                                pending.append(
                                    (
                                        gidx,
                                        (lambda s=seg_, p=par, c=ch, r=rhs,
                                         f=first_u, l=last_u: emit_av_unit(
                                            s, p, c, r, f, l, False
                                        )),
                                    )
                                )
                while pending:
                    pending.pop(0)[1]()

            LOC = list(range(NCH_LOC))
            REM = list(range(NCH_LOC, NCH))
            with (
                tc.tile_pool(name="ps_sc", bufs=2, space="PSUM") as PSC,
                tc.tile_pool(name="ps_av", bufs=1, space="PSUM") as PAV,
            ):
                emit_qproj(PSC, 0)
                run_stream(PSC, PAV, [(0, 0, LOC, "spill", "act")], hook=lambda: emit_qproj(PSC, 1))
                run_stream(PSC, PAV, [(0, 1, LOC, "spill", "act")], hook=lambda: emit_qproj(PSC, 2))
                run_stream(PSC, PAV, [(1, 0, LOC, "spill", "act")], hook=lambda: emit_qproj(PSC, 3))
                run_stream(PSC, PAV, [(1, 1, LOC, "spill", "dve")])

                # remote kv recovery: remote = (ag0 + ag1) - local  (exact)
                for h in range(2):
                    agk0 = TMP.tile([P, 512], BF16, tag="ag")
                    agk1 = TMP.tile([P, 512], BF16, tag="ag")
                    nc.sync.dma_start(agk0[:], ag_out[0, 0][:, ts(h, 512)])
                    nc.sync.dma_start(agk1[:], ag_out[1, 0][:, ts(h, 512)])
                    tk = TMP.tile([P, 512], F32, tag="th")
                    nc.vector.tensor_add(tk[:], agk0[:], agk1[:])
                    nc.vector.tensor_tensor(
                        k_rem[h][:], tk[:], k_bf[:, ts(h, 512)], ALU.subtract
                    )
                for h in range(2):
                    agv0 = TMP.tile([P, 512], BF16, tag="ag")
                    agv1 = TMP.tile([P, 512], BF16, tag="ag")
                    nc.sync.dma_start(agv0[:], ag_out[0, 1][:, ts(h, 512)])
                    nc.sync.dma_start(agv1[:], ag_out[1, 1][:, ts(h, 512)])
                    tv = TMP.tile([P, 512], F32, tag="th")
                    nc.vector.tensor_add(tv[:], agv0[:], agv1[:])
                    nc.vector.tensor_tensor(
                        v_rem[h][:, :, 0:DH],
                        tv[:].rearrange("p (a g d) -> p (a g) d", g=2, d=DH),
                        vcp_sb[:, ts(h, 4), :].rearrange("p a (g d) -> p (a g) d", g=2),
                        ALU.subtract,
                    )
                # fp8 copies of remote v, per par: v_rem[h] slot 2i+par
                for g in range(2):
                    for h in range(2):
                        nc.vector.tensor_copy(
                            v8_rem[g][:, 4 * h : 4 * h + 4, 0:DH],
                            v_rem[h][:]
                            .rearrange("p (i g2) m -> p g2 i m", g2=2)[:, g, :, 0:DH],
                        )

                run_stream(PSC, PAV, [
                    (2, 0, LOC + REM, "epi", "act"),
                    (2, 1, LOC + REM, "epi", "dve"),
                    (3, 0, LOC + REM, "epi", "act"),
                    (3, 1, LOC + REM, "epi", "act"),
                    (0, 0, REM, "epi_restore", "act"),
                    (0, 1, REM, "epi_restore", "act"),
                    (1, 0, REM, "epi_restore", "act"),
                    (1, 1, REM, "epi_restore", "dve"),
                ])

            # ---------------- phase E: output projection + residual ----------------
            with tc.tile_pool(name="ps_e", bufs=4, space="PSUM") as PE_:
                for mc in range(4):
                    for tq in range(2):
                        yps = PE_.tile([P, 512], F32, tag="yps")
                        for kc in range(4):
                            nc.tensor.matmul(
                                yps[:], wo_sb[:, kc, ts(mc, 128)],
                                ohat[kc][:, ts(tq, 512)],
                                start=(kc == 0), stop=(kc == 3),
                            )
                        yt = TMP.tile([P, 512], F32, tag="yout")
                        nc.vector.tensor_add(yt[:], yps[:], xnb[mc][:, ts(tq, 512)])
                        if TRIVIAL_BO:
                            yo = yt
                        else:
                            yo = TMP.tile([P, 512], F32, tag="yout")
                            nc.vector.tensor_scalar_add(
                                yo[:], yt[:], bo_sb[:, mc : mc + 1]
                            )
                        for dq in range(2):
                            nc.sync.dma_start(
                                out_e[mc, :, ds(tq * 512 + dq * 256, 256)],
                                yo[:, ts(dq, 256)],
                            )

    nc.compile()
    return nc


def _host_inputs(x, gamma, beta, Wq, Wkv, Wout, bout):
    """Build the 8 per-core input maps."""
    x = np.asarray(x, np.float32)
    gamma = np.asarray(gamma, np.float32)
    beta = np.asarray(beta, np.float32)
    Wq = np.asarray(Wq, np.float32)
    Wkv = np.asarray(Wkv, np.float32)
    Wout = np.asarray(Wout, np.float32)
    bout = np.asarray(bout, np.float32)

    def swap_heads(W):
        # permute output cols j -> j xor 32 within each 64-block
        c = W.shape[1]
        return np.ascontiguousarray(
            W.reshape(D, c // 64, 2, 32)[:, :, ::-1, :].reshape(D, c)
        )

    def lhsT(W):
        # [D, M] -> [128, 4, M] chunk layout
        return np.ascontiguousarray(
            W.reshape(4, P, W.shape[1]).transpose(1, 0, 2).astype(bfdt)
        )

    Wk = Wkv[:, : KVH * DH]
    Wv = Wkv[:, KVH * DH :]
    wq = lhsT(Wq)
    wqs = lhsT(swap_heads(Wq))
    wk = lhsT(Wk)
    wks = lhsT(swap_heads(Wk))
    wv = lhsT(Wv)
    wo = lhsT(Wout)
    gam = np.ascontiguousarray(gamma.reshape(4, P).T)
    bet = np.ascontiguousarray(beta.reshape(4, P).T)
    bo = np.ascontiguousarray(bout.reshape(4, P).T)
    ones = np.ones((P, 1), np.float32)

    # rotary tables (per half)
    j = np.arange(DH)
    inv_freq = 1.0 / (10000.0 ** ((2.0 * (j % 32)) / DH))
    base = ((2.0 * (j % 32)) + 0.4 * DH) / (1.4 * DH)
    sign = np.where(j < 32, -1.0, 1.0)

    tables = []
    for half in range(2):
        pos = half * NLOC + np.arange(NLOC, dtype=np.float64)
        freqs = pos[None, :] * inv_freq[:, None]  # [64, NLOC]
        cos, sin = np.cos(freqs), np.sin(freqs)
        power = (pos - N // 2) / CONTEXT_LEN
        xsc = base[:, None] ** power[None, :]
        cq = np.tile((cos * xsc), (2, 1)).astype(bfdt)
        sq = np.tile((sign[:, None] * sin * xsc), (2, 1)).astype(bfdt)
        ck = np.tile((cos / xsc), (2, 1)).astype(bfdt)
        sk = np.tile((sign[:, None] * sin / xsc), (2, 1)).astype(bfdt)
        tables.append((cq, sq, ck, sk))

    in_maps = []
    for core in range(NCORES):
        b, half = core // 2, core % 2
        xc = np.ascontiguousarray(
            x[b].reshape(4, P, N)[:, :, half * NLOC : (half + 1) * NLOC]
        )
        cq, sq, ck, sk = tables[half]
        in_maps.append(
            {
                "x": xc, "wq": wq, "wqs": wqs, "wk": wk, "wks": wks,
                "wv": wv, "wo": wo, "cq": cq, "sq": sq, "ck": ck, "sk": sk,
                "gam": gam, "bet": bet, "bo": bo, "ones": ones,
                "onesb": ones.astype(bfdt),
            }
        )
    return in_maps


def kernel(x, gamma, beta, Wq, Wkv, Wout, bout):
    trace = os.environ.get("KERNEL_TRACE", "0") == "1"
    if trace:
        _enable_trace_hook()
    trivial_gb = bool(
        np.all(np.asarray(gamma) == 1.0) and np.all(np.asarray(beta) == 0.0)
    )
    trivial_bo = bool(np.all(np.asarray(bout) == 0.0))
    if "nc" not in _CACHE:
        _CACHE["nc"] = _build(trivial_gb, trivial_bo)
        _CACHE["trivial_gb"] = (trivial_gb, trivial_bo)
    assert _CACHE["trivial_gb"] == (trivial_gb, trivial_bo)
    nc = _CACHE["nc"]
    in_maps = _host_inputs(x, gamma, beta, Wq, Wkv, Wout, bout)
    res = run_bass_kernel_spmd(nc, in_maps, list(range(NCORES)), trace=trace)
    if trace and res.exec_time_ns is not None:
        print(f"HW exec time: {res.exec_time_ns} ns")
        _CACHE["exec_time_ns"] = res.exec_time_ns

    y = np.empty((B, D, N), np.float32)
    for core in range(NCORES):
        b, half = core // 2, core % 2
        y[b, :, half * NLOC : (half + 1) * NLOC] = res.results[core]["out"].reshape(
            D, NLOC
        )
    return y


# revision 16
# speedup vs baseline: 1.0285x; 1.0285x over previous
import os
import sys
import types

import numpy as np

sys.path.insert(0, "/opt/trn_rl_repo")

import ml_dtypes  # noqa: E402
import concourse.mybir as mybir  # noqa: E402
import concourse.tile as tile  # noqa: E402
from concourse import bacc  # noqa: E402
from concourse.bass import ds, ts  # noqa: E402
from concourse.bass_utils import run_bass_kernel_spmd  # noqa: E402

BF16 = mybir.dt.bfloat16
F32 = mybir.dt.float32
FP8 = mybir.dt.float8e4
I16 = mybir.dt.int16
bfdt = ml_dtypes.bfloat16
AF = mybir.ActivationFunctionType
ALU = mybir.AluOpType
DR = mybir.MatmulPerfMode.DoubleRow

B, D, N = 4, 512, 2048
H, KVH, DH = 8, 2, 64
CONTEXT_LEN = 4096
NLOC = 1024  # tokens per core
P = 128
NCORES = 8
HP = H // 2  # head pairs
NCH = N // P  # 16 key chunks of 128
NCH_LOC = NLOC // P  # 8 local key chunks

# exp is computed as exp(0.125*s + EBIAS) so fp8e4m3 never overflows
# (max scaled score measured 8.29 -> max value ~123, NaN needs >464; fp8
# relative precision is scale-free so the extra headroom is cheap). The
# constant cancels between AV numerator and the ones-row denominator.
EBIAS = -3.4808042
# DVE Schraudolph exp: bf16 bits = floor(s*SCH_A + SCH_B) (DVE f32->i16
# convert truncates); C=6.0 calibrated for min max-rel-err (~3.7%).
SCH_A = 0.125 * 128.0 / float(np.log(2.0))
SCH_B = 16256.0 - 6.0 + EBIAS * 128.0 / float(np.log(2.0))

_CACHE = {}


def _enable_trace_hook():
    """Register the NTFF profile hook (missing antenv.axon_hooks shim)."""
    try:
        import antenv

        if "antenv.axon_hooks" in sys.modules:
            return
        mod = types.ModuleType("antenv.axon_hooks")

        def set_axon_ntff_profile_hook(h):
            mod._hook = h

        def get_axon_ntff_profile_hook():
            return getattr(mod, "_hook", None)

        mod.set_axon_ntff_profile_hook = set_axon_ntff_profile_hook
        mod.get_axon_ntff_profile_hook = get_axon_ntff_profile_hook
        sys.modules["antenv.axon_hooks"] = mod
        antenv.axon_hooks = mod
        from trn_agent_boot.trn_boot import _ntff_profile_via_ctypes

        set_axon_ntff_profile_hook(_ntff_profile_via_ctypes("/opt/axon/libaxon_pjrt.so"))
    except Exception:
        pass


def _build(TRIVIAL_GB, TRIVIAL_BO):
    nc = bacc.Bacc(None, target_bir_lowering=False, debug=False)
    dp = nc.declare_dram_parameter

    x_e = dp("x", [4, P, NLOC], F32, isOutput=False)
    wq_e = dp("wq", [P, 4, 512], BF16, isOutput=False)
    wqs_e = dp("wqs", [P, 4, 512], BF16, isOutput=False)
    wk_e = dp("wk", [P, 4, 128], BF16, isOutput=False)
    wks_e = dp("wks", [P, 4, 128], BF16, isOutput=False)
    wv_e = dp("wv", [P, 4, 128], BF16, isOutput=False)
    wo_e = dp("wo", [P, 4, 512], BF16, isOutput=False)
    cq_e = dp("cq", [P, NLOC], BF16, isOutput=False)
    sq_e = dp("sq", [P, NLOC], BF16, isOutput=False)
    ck_e = dp("ck", [P, NLOC], BF16, isOutput=False)
    sk_e = dp("sk", [P, NLOC], BF16, isOutput=False)
    gam_e = dp("gam", [P, 4], F32, isOutput=False)  # gamma per (p, chunk)
    bet_e = dp("bet", [P, 4], F32, isOutput=False)  # beta per (p, chunk)
    bo_e = dp("bo", [P, 4], F32, isOutput=False)  # bout per (p, chunk)
    ones_e = dp("ones", [P, 1], F32, isOutput=False)
    onesb_e = dp("onesb", [P, 1], BF16, isOutput=False)
    out_e = dp("out", [4, P, NLOC], F32, isOutput=True)

    with tile.TileContext(nc) as tc:
        with (
            tc.tile_pool(name="persist", bufs=1) as PS,
            tc.tile_pool(name="tmp", bufs=2) as TMP,
            tc.tile_pool(name="tmp4", bufs=4) as TMP4,
            tc.tile_pool(name="ep", bufs=2) as EP,
            tc.tile_pool(name="dram", bufs=1, space="DRAM") as DRAM,
        ):
            # ---------------- phase A: inputs -> SBUF ----------------
            SQP_cm = tc.tile_pool(name="sq_pool", bufs=1)
            SQP = SQP_cm.__enter__()
            x_sb = [
                [SQP.tile([P, 512], F32, name=f"x{c}_{tq}") for tq in range(2)]
                for c in range(4)
            ]
            for c in range(4):
                for tq in range(2):
                    nc.sync.dma_start(x_sb[c][tq][:], x_e[c][:, ts(tq, 512)])
            ones_sb = PS.tile([P, 1], F32, name="ones")
            nc.sync.dma_start(ones_sb[:], ones_e[:])
            onesb_sb = PS.tile([P, 1], BF16, name="onesb")
            nc.sync.dma_start(onesb_sb[:], onesb_e[:])
            ebias_sb = PS.tile([P, 1], F32, name="ebias")
            nc.gpsimd.memset(ebias_sb[:], EBIAS)
            gam_sb = PS.tile([P, 4], F32, name="gam")
            nc.sync.dma_start(gam_sb[:], gam_e[:])
            bet_sb = PS.tile([P, 4], F32, name="bet")
            nc.sync.dma_start(bet_sb[:], bet_e[:])
            bo_sb = PS.tile([P, 4], F32, name="bo")
            nc.sync.dma_start(bo_sb[:], bo_e[:])
            wk_sb = PS.tile([P, 4, 128], BF16, name="wk")
            nc.sync.dma_start(wk_sb[:], wk_e[:])
            wks_sb = PS.tile([P, 4, 128], BF16, name="wks")
            nc.sync.dma_start(wks_sb[:], wks_e[:])
            wv_sb = PS.tile([P, 4, 128], BF16, name="wv")
            nc.sync.dma_start(wv_sb[:], wv_e[:])
            ck_sb = PS.tile([P, NLOC], BF16, name="ck")
            nc.sync.dma_start(ck_sb[:], ck_e[:])
            sk_sb = PS.tile([P, NLOC], BF16, name="sk")
            nc.sync.dma_start(sk_sb[:], sk_e[:])
            wq_sb = PS.tile([P, 4, 512], BF16, name="wq")
            nc.sync.dma_start(wq_sb[:], wq_e[:])
            wqs_sb = PS.tile([P, 4, 512], BF16, name="wqs")
            nc.sync.dma_start(wqs_sb[:], wqs_e[:])
            cq_sb = PS.tile([P, NLOC], BF16, name="cq")
            nc.sync.dma_start(cq_sb[:], cq_e[:])
            sq_sb = PS.tile([P, NLOC], BF16, name="sq")
            nc.sync.dma_start(sq_sb[:], sq_e[:])
            wo_sb = PS.tile([P, 4, 512], BF16, name="wo")
            nc.sync.dma_start(wo_sb[:], wo_e[:])

            # v lhsT stores. bf16 copies (slot = 2*chunk + par) feed the
            # DVE-exp (Schraudolph) segments; fp8 copies (per-par, chunk
            # sequential) feed the DoubleRow segments. Col DH is the ones
            # column that accumulates the softmax denominator.
            v_loc = PS.tile([P, 2 * NCH_LOC, DH + 1], BF16, name="vloc")
            nc.gpsimd.memset(v_loc[:, :, DH : DH + 1], 1.0)
            v_rem = [
                PS.tile([P, NCH_LOC, DH + 1], BF16, name=f"vrem{h}") for h in range(2)
            ]
            for h in range(2):
                nc.gpsimd.memset(v_rem[h][:, :, DH : DH + 1], 1.0)

            xnb = [PS.tile([P, NLOC], BF16, name=f"xnb{c}") for c in range(4)]
            qr_sb = [PS.tile([P, NLOC], BF16, name=f"qr{i}") for i in range(HP)]
            k_bf = PS.tile([P, NLOC], BF16, name="kbf")
            k_rem = [PS.tile([P, 512], BF16, name=f"krem{h}") for h in range(2)]
            vcp_sb = PS.tile([P, NCH_LOC, 128], BF16, name="vcp")
            ohat = [PS.tile([P, NLOC], BF16, name=f"oh{i}") for i in range(HP)]

            ag_in = DRAM.tile([2, P, NLOC], BF16)
            ag_out = DRAM.tile([2, 2, P, NLOC], BF16)

            # ---------------- phase B: layernorm ----------------
            with tc.tile_pool(name="ps_b1", bufs=1, space="PSUM") as PB1:
                stats = PB1.tile([1, 4, 512], F32, name="stats")
                xsq = [
                    [SQP.tile([P, 512], BF16, name=f"xsq{c}_{tq}") for tq in range(2)]
                    for c in range(4)
                ]
                xb = [
                    [SQP.tile([P, 512], BF16, name=f"xb{c}_{tq}") for tq in range(2)]
                    for c in range(4)
                ]
                for c in range(4):
                    for h2 in range(2):
                        nc.vector.tensor_mul(
                            xsq[c][h2][:], x_sb[c][h2][:], x_sb[c][h2][:]
                        )
                        nc.scalar.copy(xb[c][h2][:], x_sb[c][h2][:])
                for tq in range(2):
                    for c in range(4):
                        nc.tensor.matmul(
                            stats[:, tq, :], onesb_sb[:], xb[c][tq][:],
                            start=(c == 0), stop=(c == 3),
                        )
                for tq in range(2):
                    for c in range(4):
                        nc.tensor.matmul(
                            stats[:, 2 + tq, :], onesb_sb[:], xsq[c][tq][:],
                            start=(c == 0), stop=(c == 3),
                        )
                mu_sb = TMP4.tile([1, NLOC], F32, tag="ln")
                ex2_sb = TMP4.tile([1, NLOC], F32, tag="ln")
                musq_sb = TMP4.tile([1, NLOC], F32, tag="ln")
                var_sb = TMP4.tile([1, NLOC], F32, tag="ln")
                nc.scalar.mul(mu_sb[:], stats[:, 0:2, :].rearrange("p a b -> p (a b)"), 1.0 / 512)
                nc.scalar.activation(
                    ex2_sb[:], stats[:, 2:4, :].rearrange("p a b -> p (a b)"),
                    AF.Copy, bias=1e-5, scale=1.0 / 512,
                )
                nc.vector.tensor_mul(musq_sb[:], mu_sb[:], mu_sb[:])
                nc.vector.tensor_tensor(var_sb[:], ex2_sb[:], musq_sb[:], ALU.subtract)
                # rstd = exp(-0.5 * ln(var)) -- Ln and Exp share a table set
                sd_sb = TMP4.tile([1, NLOC], F32, tag="ln")
                rstd_sb = TMP4.tile([1, NLOC], F32, tag="ln")
                nc.scalar.activation(sd_sb[:], var_sb[:], AF.Ln)
                nc.scalar.activation(rstd_sb[:], sd_sb[:], AF.Exp, scale=-0.5)
                # rstd and mu*rstd broadcast to all 128 partitions
                rstd_bc = SQP.tile([P, NLOC], F32, name="rstdbc")
                nc.gpsimd.partition_broadcast(rstd_bc[:], rstd_sb[0:1, :])
                mrs_sb = TMP4.tile([1, NLOC], F32, tag="ln")
                nc.vector.tensor_mul(mrs_sb[:], mu_sb[:], rstd_sb[:])
                mrs_bc = SQP.tile([P, NLOC], F32, name="mrsbc")
                nc.gpsimd.partition_broadcast(mrs_bc[:], mrs_sb[0:1, :])

            # xn = ((x * rstd) - mu*rstd) [* gamma + beta]   (bf16 out)
            for c in range(4):
                for tq in range(2):
                    t1 = TMP.tile([P, 512], F32, tag="th")
                    nc.vector.tensor_mul(
                        t1[:], x_sb[c][tq][:], rstd_bc[:, ts(tq, 512)]
                    )
                    if TRIVIAL_GB:
                        nc.vector.tensor_tensor(
                            xnb[c][:, ts(tq, 512)], t1[:],
                            mrs_bc[:, ts(tq, 512)], ALU.subtract,
                        )
                    else:
                        t2 = TMP.tile([P, 512], F32, tag="th")
                        nc.vector.tensor_tensor(
                            t2[:], t1[:], mrs_bc[:, ts(tq, 512)], ALU.subtract
                        )
                        nc.vector.tensor_scalar(
                            xnb[c][:, ts(tq, 512)], t2[:],
                            gam_sb[:, c : c + 1], bet_sb[:, c : c + 1],
                            ALU.mult, ALU.add,
                        )
            SQP_cm.__exit__(None, None, None)

            # ---------------- phase C1: k/v projection, rotary, allgather ----------------
            with tc.tile_pool(name="ps_c1", bufs=1, space="PSUM") as PC:
                kp0 = PC.tile([P, 2, 512], F32, name="kp0")
                kp1 = PC.tile([P, 2, 512], F32, name="kp1")
                for sw, (kps, w) in enumerate(((kp0, wk_sb), (kp1, wks_sb))):
                    for tq in range(2):
                        for c in range(4):
                            nc.tensor.matmul(
                                kps[:, tq, :], w[:, c, :], xnb[c][:, ts(tq, 512)],
                                start=(c == 0), stop=(c == 3),
                            )
                t1 = TMP.tile([P, NLOC], F32, tag="t")
                t2 = TMP.tile([P, NLOC], F32, tag="t")
                nc.vector.tensor_mul(t1[:], ck_sb[:], kp0[:].rearrange("p a b -> p (a b)"))
                nc.vector.tensor_mul(t2[:], sk_sb[:], kp1[:].rearrange("p a b -> p (a b)"))
                nc.vector.tensor_add(k_bf[:], t1[:], t2[:])

                for c8 in range(NCH_LOC):
                    vp = PC.tile([P, 128], F32, name=f"vp{c8 % 2}")
                    for c in range(4):
                        nc.tensor.matmul(
                            vp[:], xnb[c][:, ts(c8, 128)], wv_sb[:, c, :],
                            start=(c == 0), stop=(c == 3),
                        )
                    nc.vector.tensor_copy(vcp_sb[:, c8, :], vp[:])

                nc.sync.dma_start(ag_in[0], k_bf[:])
                nc.sync.dma_start(
                    ag_in[1], vcp_sb[:].rearrange("p a b -> p (a b)")
                )
                nc.gpsimd.collective_compute(
                    "AllGather",
                    ALU.bypass,
                    ins=[ag_in[:]],
                    outs=[ag_out[:]],
                    replica_groups=[[0, 1], [2, 3], [4, 5], [6, 7]],
                )

            # local v -> bf16 slots 0..15 and fp8 per-par tiles
            nc.vector.tensor_copy(
                v_loc[:, :, 0:DH],
                vcp_sb[:].rearrange("p a (g d) -> p (a g) d", g=2),
            )

            # ---------------- phase D: attention main loop ----------------
            # Per stream segment (one (hp, tq) x 8-chunk half): scores are
            # matmul'd in groups of 3 slots (slot = 2*ci+par), exp'd to an
            # E tile (slot-sequential), and AV-accumulated. ACT segments
            # produce fp8 and use DoubleRow AV over adjacent chunk pairs;
            # DVE segments produce Schraudolph bf16 (int16 bitcast) and use
            # plain bf16 AV. Denominator rides in the V ones column.
            spills = {}

            def emit_qproj(PSC, i):
                qc = TMP4.tile([P, NLOC], BF16, tag="qcs")
                qs = TMP4.tile([P, NLOC], BF16, tag="qcs")
                for tq in range(2):
                    qps = PSC.tile([P, 3, 512], F32, tag="sc", name="qps")
                    for sl, w in ((0, wq_sb), (1, wqs_sb)):
                        for c in range(4):
                            nc.tensor.matmul(
                                qps[:, sl, :], w[:, c, ts(i, 128)],
                                xnb[c][:, ts(tq, 512)],
                                start=(c == 0), stop=(c == 3),
                            )
                    nc.scalar.copy(qc[:, ts(tq, 512)], qps[:, 0, :])
                    nc.scalar.copy(qs[:, ts(tq, 512)], qps[:, 1, :])
                t1 = TMP.tile([P, NLOC], BF16, tag="qt")
                t2 = TMP.tile([P, NLOC], BF16, tag="qt")
                nc.vector.tensor_mul(t1[:], cq_sb[:], qc[:])
                nc.vector.tensor_mul(t2[:], sq_sb[:], qs[:])
                nc.vector.tensor_add(qr_sb[i][:], t1[:], t2[:])

            def emit_epilogue(hp, tq, oA, oB, restore):
                sA = TMP.tile([DH + 1, 512], F32, tag="sum")
                sB = TMP.tile([DH + 1, 512], F32, tag="sum")
                if restore:
                    cpA, cpB = spills[(hp, tq)]
                    nc.vector.tensor_add(sA[:], oA[:], cpA[:])
                    nc.vector.tensor_add(sB[:], oB[:], cpB[:])
                else:
                    nc.vector.tensor_copy(sA[:], oA[:])
                    nc.vector.tensor_copy(sB[:], oB[:])
                den2 = TMP.tile([1, 1024], F32, tag="den", bufs=1)
                nc.vector.tensor_copy(den2[0:1, 0:512], sA[DH : DH + 1, :])
                nc.vector.tensor_copy(den2[0:1, 512:1024], sB[DH : DH + 1, :])
                db = TMP.tile([64, 1024], F32, tag="db", bufs=1)
                nc.gpsimd.partition_broadcast(db[:], den2[0:1, :])
                pb = TMP.tile([64, 1024], F32, tag="pb", bufs=1)
                nc.vector.reciprocal_approx_fast(pb[:], db[:])
                nc.vector.tensor_mul(ohat[hp][0:64, ts(tq, 512)], sA[0:DH, :], pb[:, 0:512])
                nc.vector.tensor_mul(ohat[hp][64:128, ts(tq, 512)], sB[0:DH, :], pb[:, 512:1024])

            def emit_spill(hp, tq, oA, oB):
                cpA = PS.tile([DH + 1, 512], F32, name=f"spA{hp}{tq}")
                cpB = PS.tile([DH + 1, 512], F32, name=f"spB{hp}{tq}")
                nc.vector.tensor_copy(cpA[:], oA[:])
                nc.vector.tensor_copy(cpB[:], oB[:])
                spills[(hp, tq)] = (cpA, cpB)

            def run_stream(PSC, PAV, plan, hook=None):
                """plan: list of (hp, tq, chunks, mode, eng).

                chunks is a list of 8 or 16 chunk ids; eng is 'act' (fp8 +
                DoubleRow AV) or 'dve' (Schraudolph bf16 AV). Pipeline per
                3-slot group: scores | AV of ready pairs | exp.
                """
                groups = []  # (seg, half_key, hch, [slot descriptors])
                etiles = {}  # half_key -> E tile, allocated lazily at first use
                for hp, tq, chunks, mode, eng in plan:
                    nunits = 2 * len(chunks)
                    seg = {
                        "hp": hp, "tq": tq, "mode": mode, "eng": eng,
                        "nunits": nunits, "done_units": {0: 0, 1: 0},
                        "oA": None, "oB": None,
                    }
                    halves = [chunks[i : i + 8] for i in range(0, len(chunks), 8)]
                    for hi, hch in enumerate(halves):
                        hkey = (hp, tq, mode, hi)
                        slots = []
                        for ci, ch in enumerate(hch):
                            for par in range(2):
                                slots.append((par, ci, ch))
                        for gs in range(0, 16, 3):
                            groups.append((seg, hkey, hch, slots[gs : gs + 3]))

                def get_etile(seg, hkey):
                    if hkey not in etiles:
                        etiles[hkey] = EP.tile([P, 16, 512], BF16, tag="ep", name="etl")
                    return etiles[hkey]

                def emit_av_unit(seg, par, ch, rhs, first, last):
                    if first:
                        if par == 0:
                            seg["oA"] = PAV.tile([DH + 1, 512], F32, tag="avA", name="av_a")
                        else:
                            seg["oB"] = PAV.tile([DH + 1, 512], F32, tag="avB", name="av_b")
                    o = seg["oA"] if par == 0 else seg["oB"]
                    if ch < NCH_LOC:
                        vt = v_loc[:, 2 * ch + par, :]
                    else:
                        cr = ch - NCH_LOC
                        vt = v_rem[cr // 4][:, 2 * (cr % 4) + par, :]
                    nc.tensor.matmul(o[:], vt, rhs, start=first, stop=last)
                    seg["done_units"][par] += 1
                    if (
                        seg["done_units"][0] + seg["done_units"][1]
                        == seg["nunits"]
                    ):
                        oA, oB = seg["oA"], seg["oB"]
                        if seg["mode"] == "spill":
                            emit_spill(seg["hp"], seg["tq"], oA, oB)
                        else:
                            emit_epilogue(
                                seg["hp"], seg["tq"], oA, oB,
                                seg["mode"] == "epi_restore",
                            )

                pending = []  # (trigger_gidx, emit_fn)
                hook_at = len(groups) // 2
                for gidx, item in enumerate(groups + [None] * 2):
                    seg, hkey, hch, slots = item if item is not None else (None,) * 4
                    et = get_etile(seg, hkey) if seg is not None else None
                    if gidx == hook_at and hook is not None:
                        hook()
                    if seg is not None:
                        # scores for this group
                        sc = PSC.tile([P, 3, 512], F32, tag="sc")
                        hp, tq = seg["hp"], seg["tq"]
                        for pos, (par, ci, ch) in enumerate(slots):
                            if ch < NCH_LOC:
                                ksrc = k_bf[:, ts(ch, 128)]
                            else:
                                cr = ch - NCH_LOC
                                ksrc = k_rem[cr // 4][:, ts(cr % 4, 128)]
                            nc.tensor.matmul(
                                sc[:, pos, :],
                                ksrc[64 * par : 64 * (par + 1), :],
                                qr_sb[hp][64 * par : 64 * (par + 1), ts(tq, 512)],
                                start=True, stop=True,
                                tile_position=(64 * par, 0),
                            )
                    # AV matmuls whose exps are already emitted
                    while pending and pending[0][0] < gidx:
                        pending.pop(0)[1]()
                    if seg is not None:
                        # exp for this group on the segment's engine
                        ns = len(slots)
                        s0 = 2 * slots[0][1] + slots[0][0]
                        if seg["eng"] == "act":
                            nc.scalar.activation(
                                et[:, s0 : s0 + ns, :].rearrange("p a b -> p (a b)"),
                                sc[:, 0:ns, :].rearrange("p a b -> p (a b)"),
                                AF.Exp, bias=ebias_sb[:], scale=0.125,
                            )
                        else:
                            nc.vector.tensor_scalar(
                                et[:, s0 : s0 + ns, :]
                                .rearrange("p a b -> p (a b)")
                                .bitcast(I16),
                                sc[:, 0:ns, :].rearrange("p a b -> p (a b)"),
                                SCH_A, SCH_B, ALU.mult, ALU.add,
                            )
                        # register completed AV units
                        for par, ci, ch in slots:
                            seg_, et_ = seg, et
                            rhs = et_[:, 2 * ci + par, :]
                            nu = seg_["sched_units"] = seg_.get("sched_units", {0: 0, 1: 0})
                            nu[par] += 1
                            first_u = nu[par] == 1
                            last_u = nu[par] == seg_["nunits"] // 2
                            pending.append(
                                (
                                    gidx,
                                    (lambda s=seg_, p=par, c=ch, r=rhs,
                                     f=first_u, l=last_u: emit_av_unit(
                                        s, p, c, r, f, l
                                    )),
                                )
                            )
                while pending:
                    pending.pop(0)[1]()

            LOC = list(range(NCH_LOC))
            REM = list(range(NCH_LOC, NCH))
            with (
                tc.tile_pool(name="ps_sc", bufs=2, space="PSUM") as PSC,
                tc.tile_pool(name="ps_av", bufs=1, space="PSUM") as PAV,
            ):
                emit_qproj(PSC, 0)
                run_stream(PSC, PAV, [(0, 0, LOC, "spill", "act")], hook=lambda: emit_qproj(PSC, 1))
                run_stream(PSC, PAV, [(0, 1, LOC, "spill", "act")], hook=lambda: emit_qproj(PSC, 2))
                run_stream(PSC, PAV, [(1, 0, LOC, "spill", "act")], hook=lambda: emit_qproj(PSC, 3))
                run_stream(PSC, PAV, [(1, 1, LOC, "spill", "dve")])

                # remote kv recovery: remote = (ag0 + ag1) - local  (exact)
                for h in range(2):
                    agk0 = TMP.tile([P, 512], BF16, tag="ag")
                    agk1 = TMP.tile([P, 512], BF16, tag="ag")
                    nc.sync.dma_start(agk0[:], ag_out[0, 0][:, ts(h, 512)])
                    nc.sync.dma_start(agk1[:], ag_out[1, 0][:, ts(h, 512)])
                    tk = TMP.tile([P, 512], F32, tag="th")
                    nc.vector.tensor_add(tk[:], agk0[:], agk1[:])
                    nc.vector.tensor_tensor(
                        k_rem[h][:], tk[:], k_bf[:, ts(h, 512)], ALU.subtract
                    )
                for h in range(2):
                    agv0 = TMP.tile([P, 512], BF16, tag="ag")
                    agv1 = TMP.tile([P, 512], BF16, tag="ag")
                    nc.sync.dma_start(agv0[:], ag_out[0, 1][:, ts(h, 512)])
                    nc.sync.dma_start(agv1[:], ag_out[1, 1][:, ts(h, 512)])
                    tv = TMP.tile([P, 512], F32, tag="th")
                    nc.vector.tensor_add(tv[:], agv0[:], agv1[:])
                    nc.vector.tensor_tensor(
                        v_rem[h][:, :, 0:DH],
                        tv[:].rearrange("p (a g d) -> p (a g) d", g=2, d=DH),
                        vcp_sb[:, ts(h, 4), :].rearrange("p a (g d) -> p (a g) d", g=2),
                        ALU.subtract,
                    )

                run_stream(PSC, PAV, [
                    (2, 0, LOC + REM, "epi", "act"),
                    (2, 1, LOC + REM, "epi", "dve"),
                    (3, 0, LOC + REM, "epi", "act"),
                    (3, 1, LOC + REM, "epi", "act"),
                    (0, 0, REM, "epi_restore", "act"),
                    (0, 1, REM, "epi_restore", "act"),
                    (1, 0, REM, "epi_restore", "act"),
                    (1, 1, REM, "epi_restore", "dve"),
                ])

            # ---------------- phase E: output projection + residual ----------------
            with tc.tile_pool(name="ps_e", bufs=4, space="PSUM") as PE_:
                for mc in range(4):
                    for tq in range(2):
                        yps = PE_.tile([P, 512], F32, tag="yps")
                        for kc in range(4):
                            nc.tensor.matmul(
                                yps[:], wo_sb[:, kc, ts(mc, 128)],
                                ohat[kc][:, ts(tq, 512)],
                                start=(kc == 0), stop=(kc == 3),
                            )
                        yt = TMP.tile([P, 512], F32, tag="yout")
                        nc.vector.tensor_add(yt[:], yps[:], xnb[mc][:, ts(tq, 512)])
                        if TRIVIAL_BO:
                            yo = yt
                        else:
                            yo = TMP.tile([P, 512], F32, tag="yout")
                            nc.vector.tensor_scalar_add(
                                yo[:], yt[:], bo_sb[:, mc : mc + 1]
                            )
                        for dq in range(2):
                            nc.sync.dma_start(
                                out_e[mc, :, ds(tq * 512 + dq * 256, 256)],
                                yo[:, ts(dq, 256)],
                            )

    nc.compile()
    return nc


def _host_inputs(x, gamma, beta, Wq, Wkv, Wout, bout):
    """Build the 8 per-core input maps."""
    x = np.asarray(x, np.float32)
    gamma = np.asarray(gamma, np.float32)
    beta = np.asarray(beta, np.float32)
    Wq = np.asarray(Wq, np.float32)
    Wkv = np.asarray(Wkv, np.float32)
    Wout = np.asarray(Wout, np.float32)
    bout = np.asarray(bout, np.float32)

    def swap_heads(W):
        # permute output cols j -> j xor 32 within each 64-block
        c = W.shape[1]
        return np.ascontiguousarray(
            W.reshape(D, c // 64, 2, 32)[:, :, ::-1, :].reshape(D, c)
        )

    def lhsT(W):
        # [D, M] -> [128, 4, M] chunk layout
        return np.ascontiguousarray(
            W.reshape(4, P, W.shape[1]).transpose(1, 0, 2).astype(bfdt)
        )

    Wk = Wkv[:, : KVH * DH]
    Wv = Wkv[:, KVH * DH :]
    wq = lhsT(Wq)
    wqs = lhsT(swap_heads(Wq))
    wk = lhsT(Wk)
    wks = lhsT(swap_heads(Wk))
    wv = lhsT(Wv)
    wo = lhsT(Wout)
    gam = np.ascontiguousarray(gamma.reshape(4, P).T)
    bet = np.ascontiguousarray(beta.reshape(4, P).T)
    bo = np.ascontiguousarray(bout.reshape(4, P).T)
    ones = np.ones((P, 1), np.float32)

    # rotary tables (per half)
    j = np.arange(DH)
    inv_freq = 1.0 / (10000.0 ** ((2.0 * (j % 32)) / DH))
    base = ((2.0 * (j % 32)) + 0.4 * DH) / (1.4 * DH)
    sign = np.where(j < 32, -1.0, 1.0)

    tables = []
    for half in range(2):
        pos = half * NLOC + np.arange(NLOC, dtype=np.float64)
        freqs = pos[None, :] * inv_freq[:, None]  # [64, NLOC]
        cos, sin = np.cos(freqs), np.sin(freqs)
        power = (pos - N // 2) / CONTEXT_LEN
        xsc = base[:, None] ** power[None, :]
        cq = np.tile((cos * xsc), (2, 1)).astype(bfdt)
        sq = np.tile((sign[:, None] * sin * xsc), (2, 1)).astype(bfdt)
        ck = np.tile((cos / xsc), (2, 1)).astype(bfdt)
        sk = np.tile((sign[:, None] * sin / xsc), (2, 1)).astype(bfdt)
        tables.append((cq, sq, ck, sk))

    in_maps = []
    for core in range(NCORES):
        b, half = core // 2, core % 2
        xc = np.ascontiguousarray(
            x[b].reshape(4, P, N)[:, :, half * NLOC : (half + 1) * NLOC]
        )
        cq, sq, ck, sk = tables[half]
        in_maps.append(
            {
                "x": xc, "wq": wq, "wqs": wqs, "wk": wk, "wks": wks,
                "wv": wv, "wo": wo, "cq": cq, "sq": sq, "ck": ck, "sk": sk,
                "gam": gam, "bet": bet, "bo": bo, "ones": ones,
                "onesb": ones.astype(bfdt),
            }
        )
    return in_maps


def kernel(x, gamma, beta, Wq, Wkv, Wout, bout):
    trace = os.environ.get("KERNEL_TRACE", "0") == "1"
    if trace:
        _enable_trace_hook()
    trivial_gb = bool(
        np.all(np.asarray(gamma) == 1.0) and np.all(np.asarray(beta) == 0.0)
    )
    trivial_bo = bool(np.all(np.asarray(bout) == 0.0))
    if "nc" not in _CACHE:
        _CACHE["nc"] = _build(trivial_gb, trivial_bo)
        _CACHE["trivial_gb"] = (trivial_gb, trivial_bo)
    assert _CACHE["trivial_gb"] == (trivial_gb, trivial_bo)
    nc = _CACHE["nc"]
    in_maps = _host_inputs(x, gamma, beta, Wq, Wkv, Wout, bout)
    res = run_bass_kernel_spmd(nc, in_maps, list(range(NCORES)), trace=trace)
    if trace and res.exec_time_ns is not None:
        print(f"HW exec time: {res.exec_time_ns} ns")
        _CACHE["exec_time_ns"] = res.exec_time_ns

    y = np.empty((B, D, N), np.float32)
    for core in range(NCORES):
        b, half = core // 2, core % 2
        y[b, :, half * NLOC : (half + 1) * NLOC] = res.results[core]["out"].reshape(
            D, NLOC
        )
    return y


# revision 17
# speedup vs baseline: 1.2375x; 1.2032x over previous
import os
import sys
import types

import numpy as np

sys.path.insert(0, "/opt/trn_rl_repo")

import ml_dtypes  # noqa: E402
import concourse.mybir as mybir  # noqa: E402
import concourse.tile as tile  # noqa: E402
from concourse import bacc  # noqa: E402
from concourse.bass import ds, ts  # noqa: E402
from concourse.bass_utils import run_bass_kernel_spmd  # noqa: E402

BF16 = mybir.dt.bfloat16
F32 = mybir.dt.float32
FP8 = mybir.dt.float8e4
I16 = mybir.dt.int16
bfdt = ml_dtypes.bfloat16
AF = mybir.ActivationFunctionType
ALU = mybir.AluOpType
DR = mybir.MatmulPerfMode.DoubleRow

B, D, N = 4, 512, 2048
H, KVH, DH = 8, 2, 64
CONTEXT_LEN = 4096
NLOC = 1024  # tokens per core
P = 128
NCORES = 8
HP = H // 2  # head pairs
NCH = N // P  # 16 key chunks of 128
NCH_LOC = NLOC // P  # 8 local key chunks

# DVE Schraudolph exp: bf16 bits = floor(s*SCH_A + SCH_B) (DVE f32->i16
# convert truncates); C=6.0 calibrated for min max-rel-err (~3.7%).
# Scores*0.125 measured in [-8.8, 8.3] so bits stay in [14633, 17780].
SCH_A = 0.125 * 128.0 / float(np.log(2.0))
SCH_B = 16256.0 - 6.0

_CACHE = {}


def _enable_trace_hook():
    """Register the NTFF profile hook (missing antenv.axon_hooks shim)."""
    try:
        import antenv

        if "antenv.axon_hooks" in sys.modules:
            return
        mod = types.ModuleType("antenv.axon_hooks")

        def set_axon_ntff_profile_hook(h):
            mod._hook = h

        def get_axon_ntff_profile_hook():
            return getattr(mod, "_hook", None)

        mod.set_axon_ntff_profile_hook = set_axon_ntff_profile_hook
        mod.get_axon_ntff_profile_hook = get_axon_ntff_profile_hook
        sys.modules["antenv.axon_hooks"] = mod
        antenv.axon_hooks = mod
        from trn_agent_boot.trn_boot import _ntff_profile_via_ctypes

        set_axon_ntff_profile_hook(_ntff_profile_via_ctypes("/opt/axon/libaxon_pjrt.so"))
    except Exception:
        pass


def _build(TRIVIAL_GB, TRIVIAL_BO):
    nc = bacc.Bacc(None, target_bir_lowering=False, debug=False)
    dp = nc.declare_dram_parameter

    x_e = dp("x", [4, P, NLOC], F32, isOutput=False)
    wq_e = dp("wq", [P, 4, 512], BF16, isOutput=False)
    wqs_e = dp("wqs", [P, 4, 512], BF16, isOutput=False)
    wk_e = dp("wk", [P, 4, 128], BF16, isOutput=False)
    wks_e = dp("wks", [P, 4, 128], BF16, isOutput=False)
    wv_e = dp("wv", [P, 4, 128], BF16, isOutput=False)
    wo_e = dp("wo", [P, 4, 512], BF16, isOutput=False)
    cq_e = dp("cq", [P, NLOC], BF16, isOutput=False)
    sq_e = dp("sq", [P, NLOC], BF16, isOutput=False)
    ck_e = dp("ck", [P, NLOC], BF16, isOutput=False)
    sk_e = dp("sk", [P, NLOC], BF16, isOutput=False)
    gam_e = dp("gam", [P, 4], F32, isOutput=False)  # gamma per (p, chunk)
    bet_e = dp("bet", [P, 4], F32, isOutput=False)  # beta per (p, chunk)
    bo_e = dp("bo", [P, 4], F32, isOutput=False)  # bout per (p, chunk)
    ones_e = dp("ones", [P, 1], F32, isOutput=False)
    onesb_e = dp("onesb", [P, 1], BF16, isOutput=False)
    out_e = dp("out", [4, P, NLOC], F32, isOutput=True)

    with tile.TileContext(nc) as tc:
        with (
            tc.tile_pool(name="persist", bufs=1) as PS,
            tc.tile_pool(name="tmp", bufs=2) as TMP,
            tc.tile_pool(name="tmp4", bufs=4) as TMP4,
            tc.tile_pool(name="ep", bufs=3) as EP,
            tc.tile_pool(name="dram", bufs=1, space="DRAM") as DRAM,
        ):
            # ---------------- phase A: inputs -> SBUF ----------------
            SQP_cm = tc.tile_pool(name="sq_pool", bufs=1)
            SQP = SQP_cm.__enter__()
            x_sb = [
                [SQP.tile([P, 512], F32, name=f"x{c}_{tq}") for tq in range(2)]
                for c in range(4)
            ]
            for c in range(4):
                for tq in range(2):
                    nc.sync.dma_start(x_sb[c][tq][:], x_e[c][:, ts(tq, 512)])
            ones_sb = PS.tile([P, 1], F32, name="ones")
            nc.sync.dma_start(ones_sb[:], ones_e[:])
            onesb_sb = PS.tile([P, 1], BF16, name="onesb")
            nc.sync.dma_start(onesb_sb[:], onesb_e[:])
            gam_sb = PS.tile([P, 4], F32, name="gam")
            nc.sync.dma_start(gam_sb[:], gam_e[:])
            bet_sb = PS.tile([P, 4], F32, name="bet")
            nc.sync.dma_start(bet_sb[:], bet_e[:])
            bo_sb = PS.tile([P, 4], F32, name="bo")
            nc.sync.dma_start(bo_sb[:], bo_e[:])
            wk_sb = PS.tile([P, 4, 128], BF16, name="wk")
            nc.sync.dma_start(wk_sb[:], wk_e[:])
            wks_sb = PS.tile([P, 4, 128], BF16, name="wks")
            nc.sync.dma_start(wks_sb[:], wks_e[:])
            wv_sb = PS.tile([P, 4, 128], BF16, name="wv")
            nc.sync.dma_start(wv_sb[:], wv_e[:])
            ck_sb = PS.tile([P, NLOC], BF16, name="ck")
            nc.sync.dma_start(ck_sb[:], ck_e[:])
            sk_sb = PS.tile([P, NLOC], BF16, name="sk")
            nc.sync.dma_start(sk_sb[:], sk_e[:])
            wq_sb = PS.tile([P, 4, 512], BF16, name="wq")
            nc.sync.dma_start(wq_sb[:], wq_e[:])
            wqs_sb = PS.tile([P, 4, 512], BF16, name="wqs")
            nc.sync.dma_start(wqs_sb[:], wqs_e[:])
            cq_sb = PS.tile([P, NLOC], BF16, name="cq")
            nc.sync.dma_start(cq_sb[:], cq_e[:])
            sq_sb = PS.tile([P, NLOC], BF16, name="sq")
            nc.sync.dma_start(sq_sb[:], sq_e[:])
            wo_sb = PS.tile([P, 4, 512], BF16, name="wo")
            nc.sync.dma_start(wo_sb[:], wo_e[:])

            # v lhsT stores. bf16 copies (slot = 2*chunk + par) feed the
            # DVE-exp (Schraudolph) segments; fp8 copies (per-par, chunk
            # sequential) feed the DoubleRow segments. Col DH is the ones
            # column that accumulates the softmax denominator.
            v_loc = PS.tile([P, 2 * NCH_LOC, DH + 1], BF16, name="vloc")
            nc.gpsimd.memset(v_loc[:, :, DH : DH + 1], 1.0)
            v_rem = [
                PS.tile([P, NCH_LOC, DH + 1], BF16, name=f"vrem{h}") for h in range(2)
            ]
            for h in range(2):
                nc.gpsimd.memset(v_rem[h][:, :, DH : DH + 1], 1.0)

            xnb = [PS.tile([P, NLOC], BF16, name=f"xnb{c}") for c in range(4)]
            qr_sb = [PS.tile([P, NLOC], BF16, name=f"qr{i}") for i in range(HP)]
            k_bf = PS.tile([P, NLOC], BF16, name="kbf")
            k_rem = [PS.tile([P, 512], BF16, name=f"krem{h}") for h in range(2)]
            vcp_sb = PS.tile([P, NCH_LOC, 128], BF16, name="vcp")
            ohat = [PS.tile([P, NLOC], BF16, name=f"oh{i}") for i in range(HP)]

            ag_in = DRAM.tile([2, P, NLOC], BF16)
            ag_out = DRAM.tile([2, 2, P, NLOC], BF16)

            # ---------------- phase B: layernorm ----------------
            with tc.tile_pool(name="ps_b1", bufs=1, space="PSUM") as PB1:
                stats = PB1.tile([1, 4, 512], F32, name="stats")
                xsq = [
                    [SQP.tile([P, 512], BF16, name=f"xsq{c}_{tq}") for tq in range(2)]
                    for c in range(4)
                ]
                xb = [
                    [SQP.tile([P, 512], BF16, name=f"xb{c}_{tq}") for tq in range(2)]
                    for c in range(4)
                ]
                for c in range(4):
                    for h2 in range(2):
                        nc.vector.tensor_mul(
                            xsq[c][h2][:], x_sb[c][h2][:], x_sb[c][h2][:]
                        )
                        nc.scalar.copy(xb[c][h2][:], x_sb[c][h2][:])
                for tq in range(2):
                    for c in range(4):
                        nc.tensor.matmul(
                            stats[:, tq, :], onesb_sb[:], xb[c][tq][:],
                            start=(c == 0), stop=(c == 3),
                        )
                for tq in range(2):
                    for c in range(4):
                        nc.tensor.matmul(
                            stats[:, 2 + tq, :], onesb_sb[:], xsq[c][tq][:],
                            start=(c == 0), stop=(c == 3),
                        )
                mu_sb = TMP4.tile([1, NLOC], F32, tag="ln")
                ex2_sb = TMP4.tile([1, NLOC], F32, tag="ln")
                musq_sb = TMP4.tile([1, NLOC], F32, tag="ln")
                var_sb = TMP4.tile([1, NLOC], F32, tag="ln")
                nc.scalar.mul(mu_sb[:], stats[:, 0:2, :].rearrange("p a b -> p (a b)"), 1.0 / 512)
                nc.scalar.activation(
                    ex2_sb[:], stats[:, 2:4, :].rearrange("p a b -> p (a b)"),
                    AF.Copy, bias=1e-5, scale=1.0 / 512,
                )
                nc.vector.tensor_mul(musq_sb[:], mu_sb[:], mu_sb[:])
                nc.vector.tensor_tensor(var_sb[:], ex2_sb[:], musq_sb[:], ALU.subtract)
                # rstd = exp(-0.5 * ln(var)) -- Ln and Exp share a table set
                sd_sb = TMP4.tile([1, NLOC], F32, tag="ln")
                rstd_sb = TMP4.tile([1, NLOC], F32, tag="ln")
                nc.scalar.activation(sd_sb[:], var_sb[:], AF.Ln)
                nc.scalar.activation(rstd_sb[:], sd_sb[:], AF.Exp, scale=-0.5)
                # rstd and mu*rstd broadcast to all 128 partitions
                rstd_bc = SQP.tile([P, NLOC], F32, name="rstdbc")
                nc.gpsimd.partition_broadcast(rstd_bc[:], rstd_sb[0:1, :])
                mrs_sb = TMP4.tile([1, NLOC], F32, tag="ln")
                nc.vector.tensor_mul(mrs_sb[:], mu_sb[:], rstd_sb[:])
                mrs_bc = SQP.tile([P, NLOC], F32, name="mrsbc")
                nc.gpsimd.partition_broadcast(mrs_bc[:], mrs_sb[0:1, :])

            # xn = ((x * rstd) - mu*rstd) [* gamma + beta]   (bf16 out)
            for c in range(4):
                for tq in range(2):
                    t1 = TMP.tile([P, 512], F32, tag="th")
                    nc.vector.tensor_mul(
                        t1[:], x_sb[c][tq][:], rstd_bc[:, ts(tq, 512)]
                    )
                    if TRIVIAL_GB:
                        nc.vector.tensor_tensor(
                            xnb[c][:, ts(tq, 512)], t1[:],
                            mrs_bc[:, ts(tq, 512)], ALU.subtract,
                        )
                    else:
                        t2 = TMP.tile([P, 512], F32, tag="th")
                        nc.vector.tensor_tensor(
                            t2[:], t1[:], mrs_bc[:, ts(tq, 512)], ALU.subtract
                        )
                        nc.vector.tensor_scalar(
                            xnb[c][:, ts(tq, 512)], t2[:],
                            gam_sb[:, c : c + 1], bet_sb[:, c : c + 1],
                            ALU.mult, ALU.add,
                        )
            SQP_cm.__exit__(None, None, None)

            # ---------------- phase C1: k/v projection, rotary, allgather ----------------
            with tc.tile_pool(name="ps_c1", bufs=1, space="PSUM") as PC:
                kp0 = PC.tile([P, 2, 512], F32, name="kp0")
                kp1 = PC.tile([P, 2, 512], F32, name="kp1")
                for sw, (kps, w) in enumerate(((kp0, wk_sb), (kp1, wks_sb))):
                    for tq in range(2):
                        for c in range(4):
                            nc.tensor.matmul(
                                kps[:, tq, :], w[:, c, :], xnb[c][:, ts(tq, 512)],
                                start=(c == 0), stop=(c == 3),
                            )
                t1 = TMP.tile([P, NLOC], F32, tag="t")
                t2 = TMP.tile([P, NLOC], F32, tag="t")
                nc.vector.tensor_mul(t1[:], ck_sb[:], kp0[:].rearrange("p a b -> p (a b)"))
                nc.vector.tensor_mul(t2[:], sk_sb[:], kp1[:].rearrange("p a b -> p (a b)"))
                nc.vector.tensor_add(k_bf[:], t1[:], t2[:])

                for c8 in range(NCH_LOC):
                    vp = PC.tile([P, 128], F32, name=f"vp{c8 % 2}")
                    for c in range(4):
                        nc.tensor.matmul(
                            vp[:], xnb[c][:, ts(c8, 128)], wv_sb[:, c, :],
                            start=(c == 0), stop=(c == 3),
                        )
                    nc.vector.tensor_copy(vcp_sb[:, c8, :], vp[:])

                nc.sync.dma_start(ag_in[0], k_bf[:])
                nc.sync.dma_start(
                    ag_in[1], vcp_sb[:].rearrange("p a b -> p (a b)")
                )
                nc.gpsimd.collective_compute(
                    "AllGather",
                    ALU.bypass,
                    ins=[ag_in[:]],
                    outs=[ag_out[:]],
                    replica_groups=[[0, 1], [2, 3], [4, 5], [6, 7]],
                )

            # local v -> bf16 slots 0..15 and fp8 per-par tiles
            nc.vector.tensor_copy(
                v_loc[:, :, 0:DH],
                vcp_sb[:].rearrange("p a (g d) -> p (a g) d", g=2),
            )

            # ---------------- phase D: attention main loop ----------------
            # Per stream segment (one (hp, tq) x 8-chunk half): scores are
            # matmul'd in groups of 3 slots (slot = 2*ci+par), exp'd to an
            # E tile (slot-sequential), and AV-accumulated. ACT segments
            # produce fp8 and use DoubleRow AV over adjacent chunk pairs;
            # DVE segments produce Schraudolph bf16 (int16 bitcast) and use
            # plain bf16 AV. Denominator rides in the V ones column.
            spills = {}

            def emit_qproj(PSC, i):
                qc = TMP4.tile([P, NLOC], BF16, tag="qcs")
                qs = TMP4.tile([P, NLOC], BF16, tag="qcs")
                for tq in range(2):
                    qps = PSC.tile([P, 3, 512], F32, tag="sc", name="qps")
                    for sl, w in ((0, wq_sb), (1, wqs_sb)):
                        for c in range(4):
                            nc.tensor.matmul(
                                qps[:, sl, :], w[:, c, ts(i, 128)],
                                xnb[c][:, ts(tq, 512)],
                                start=(c == 0), stop=(c == 3),
                            )
                    nc.scalar.copy(qc[:, ts(tq, 512)], qps[:, 0, :])
                    nc.scalar.copy(qs[:, ts(tq, 512)], qps[:, 1, :])
                t1 = TMP.tile([P, NLOC], BF16, tag="qt")
                t2 = TMP.tile([P, NLOC], BF16, tag="qt")
                nc.vector.tensor_mul(t1[:], cq_sb[:], qc[:])
                nc.vector.tensor_mul(t2[:], sq_sb[:], qs[:])
                nc.vector.tensor_add(qr_sb[i][:], t1[:], t2[:])

            def emit_epilogue(hp, tq, oA, oB, restore):
                sA = TMP.tile([DH + 1, 512], F32, tag="sum")
                sB = TMP.tile([DH + 1, 512], F32, tag="sum")
                if restore:
                    cpA, cpB = spills[(hp, tq)]
                    nc.vector.tensor_add(sA[:], oA[:], cpA[:])
                    nc.vector.tensor_add(sB[:], oB[:], cpB[:])
                else:
                    nc.vector.tensor_copy(sA[:], oA[:])
                    nc.vector.tensor_copy(sB[:], oB[:])
                den2 = TMP.tile([1, 1024], F32, tag="den", bufs=1)
                nc.vector.tensor_copy(den2[0:1, 0:512], sA[DH : DH + 1, :])
                nc.vector.tensor_copy(den2[0:1, 512:1024], sB[DH : DH + 1, :])
                db = TMP.tile([64, 1024], F32, tag="db", bufs=1)
                nc.gpsimd.partition_broadcast(db[:], den2[0:1, :])
                pb = TMP.tile([64, 1024], F32, tag="pb", bufs=1)
                nc.vector.reciprocal_approx_fast(pb[:], db[:])
                nc.vector.tensor_mul(ohat[hp][0:64, ts(tq, 512)], sA[0:DH, :], pb[:, 0:512])
                nc.vector.tensor_mul(ohat[hp][64:128, ts(tq, 512)], sB[0:DH, :], pb[:, 512:1024])

            def emit_spill(hp, tq, oA, oB):
                cpA = PS.tile([DH + 1, 512], F32, name=f"spA{hp}{tq}")
                cpB = PS.tile([DH + 1, 512], F32, name=f"spB{hp}{tq}")
                nc.vector.tensor_copy(cpA[:], oA[:])
                nc.vector.tensor_copy(cpB[:], oB[:])
                spills[(hp, tq)] = (cpA, cpB)

            def run_stream(PSC, PAV, plan, hook=None):
                """plan: list of (hp, tq, chunks, mode, eng).

                chunks is a list of 8 or 16 chunk ids; eng is 'act' (fp8 +
                DoubleRow AV) or 'dve' (Schraudolph bf16 AV). Pipeline per
                3-slot group: scores | AV of ready pairs | exp.
                """
                groups = []
                for hp, tq, chunks, mode, eng in plan:
                    nunits = 2 * len(chunks)
                    seg = {
                        "hp": hp, "tq": tq, "mode": mode, "eng": eng,
                        "nunits": nunits, "done_units": {0: 0, 1: 0},
                        "oA": None, "oB": None,
                    }
                    slots = []
                    for ci, ch in enumerate(chunks):
                        for par in range(2):
                            slots.append((par, ci, ch))
                    for gs in range(0, len(slots), 3):
                        groups.append((seg, slots[gs : gs + 3]))

                def emit_av_unit(seg, par, ch, rhs, first, last):
                    if first:
                        if par == 0:
                            seg["oA"] = PAV.tile([DH + 1, 512], F32, tag="avA", name="av_a")
                        else:
                            seg["oB"] = PAV.tile([DH + 1, 512], F32, tag="avB", name="av_b")
                    o = seg["oA"] if par == 0 else seg["oB"]
                    if ch < NCH_LOC:
                        vt = v_loc[:, 2 * ch + par, :]
                    else:
                        cr = ch - NCH_LOC
                        vt = v_rem[cr // 4][:, 2 * (cr % 4) + par, :]
                    nc.tensor.matmul(o[:], vt, rhs, start=first, stop=last)
                    seg["done_units"][par] += 1
                    if (
                        seg["done_units"][0] + seg["done_units"][1]
                        == seg["nunits"]
                    ):
                        oA, oB = seg["oA"], seg["oB"]
                        if seg["mode"] == "spill":
                            emit_spill(seg["hp"], seg["tq"], oA, oB)
                        else:
                            emit_epilogue(
                                seg["hp"], seg["tq"], oA, oB,
                                seg["mode"] == "epi_restore",
                            )

                pending = []  # (trigger_gidx, emit_fn)
                hook_at = len(groups) // 2
                for gidx, item in enumerate(groups + [None] * 2):
                    seg, slots = item if item is not None else (None, None)
                    if gidx == hook_at and hook is not None:
                        hook()
                    if seg is not None:
                        # scores for this group
                        sc = PSC.tile([P, 3, 512], F32, tag="sc")
                        hp, tq = seg["hp"], seg["tq"]
                        for pos, (par, ci, ch) in enumerate(slots):
                            if ch < NCH_LOC:
                                ksrc = k_bf[:, ts(ch, 128)]
                            else:
                                cr = ch - NCH_LOC
                                ksrc = k_rem[cr // 4][:, ts(cr % 4, 128)]
                            nc.tensor.matmul(
                                sc[:, pos, :],
                                ksrc[64 * par : 64 * (par + 1), :],
                                qr_sb[hp][64 * par : 64 * (par + 1), ts(tq, 512)],
                                start=True, stop=True,
                                tile_position=(64 * par, 0),
                            )
                    # AV matmuls whose exps are already emitted
                    while pending and pending[0][0] < gidx:
                        pending.pop(0)[1]()
                    if seg is not None:
                        # exp for this group on the segment's engine
                        ns = len(slots)
                        et = EP.tile([P, 3, 512], BF16, tag="ep", name="etg")
                        if seg["eng"] == "act":
                            nc.scalar.activation(
                                et[:, 0:ns, :].rearrange("p a b -> p (a b)"),
                                sc[:, 0:ns, :].rearrange("p a b -> p (a b)"),
                                AF.Exp, scale=0.125,
                            )
                        else:
                            nc.vector.tensor_scalar(
                                et[:, 0:ns, :]
                                .rearrange("p a b -> p (a b)")
                                .bitcast(I16),
                                sc[:, 0:ns, :].rearrange("p a b -> p (a b)"),
                                SCH_A, SCH_B, ALU.mult, ALU.add,
                            )
                        # register completed AV units
                        for pos, (par, ci, ch) in enumerate(slots):
                            seg_ = seg
                            rhs = et[:, pos, :]
                            nu = seg_["sched_units"] = seg_.get("sched_units", {0: 0, 1: 0})
                            nu[par] += 1
                            first_u = nu[par] == 1
                            last_u = nu[par] == seg_["nunits"] // 2
                            pending.append(
                                (
                                    gidx,
                                    (lambda s=seg_, p=par, c=ch, r=rhs,
                                     f=first_u, l=last_u: emit_av_unit(
                                        s, p, c, r, f, l
                                    )),
                                )
                            )
                while pending:
                    pending.pop(0)[1]()

            LOC = list(range(NCH_LOC))
            REM = list(range(NCH_LOC, NCH))
            with (
                tc.tile_pool(name="ps_sc", bufs=2, space="PSUM") as PSC,
                tc.tile_pool(name="ps_av", bufs=1, space="PSUM") as PAV,
            ):
                emit_qproj(PSC, 0)
                run_stream(PSC, PAV, [(0, 0, LOC, "spill", "act")], hook=lambda: emit_qproj(PSC, 1))
                run_stream(PSC, PAV, [(0, 1, LOC, "spill", "act")], hook=lambda: emit_qproj(PSC, 2))
                run_stream(PSC, PAV, [(1, 0, LOC, "spill", "act")], hook=lambda: emit_qproj(PSC, 3))
                run_stream(PSC, PAV, [(1, 1, LOC, "spill", "dve")])

                # remote kv recovery: remote = (ag0 + ag1) - local  (exact)
                for h in range(2):
                    agk0 = TMP.tile([P, 512], BF16, tag="ag")
                    agk1 = TMP.tile([P, 512], BF16, tag="ag")
                    nc.sync.dma_start(agk0[:], ag_out[0, 0][:, ts(h, 512)])
                    nc.sync.dma_start(agk1[:], ag_out[1, 0][:, ts(h, 512)])
                    tk = TMP.tile([P, 512], F32, tag="th")
                    nc.vector.tensor_add(tk[:], agk0[:], agk1[:])
                    nc.vector.tensor_tensor(
                        k_rem[h][:], tk[:], k_bf[:, ts(h, 512)], ALU.subtract
                    )
                for h in range(2):
                    agv0 = TMP.tile([P, 512], BF16, tag="ag")
                    agv1 = TMP.tile([P, 512], BF16, tag="ag")
                    nc.sync.dma_start(agv0[:], ag_out[0, 1][:, ts(h, 512)])
                    nc.sync.dma_start(agv1[:], ag_out[1, 1][:, ts(h, 512)])
                    tv = TMP.tile([P, 512], F32, tag="th")
                    nc.vector.tensor_add(tv[:], agv0[:], agv1[:])
                    nc.vector.tensor_tensor(
                        v_rem[h][:, :, 0:DH],
                        tv[:].rearrange("p (a g d) -> p (a g) d", g=2, d=DH),
                        vcp_sb[:, ts(h, 4), :].rearrange("p a (g d) -> p (a g) d", g=2),
                        ALU.subtract,
                    )

                run_stream(PSC, PAV, [
                    (2, 0, LOC + REM, "epi", "act"),
                    (2, 1, LOC + REM, "epi", "dve"),
                    (3, 0, LOC + REM, "epi", "act"),
                    (3, 1, LOC + REM, "epi", "act"),
                    (0, 0, REM, "epi_restore", "act"),
                    (0, 1, REM, "epi_restore", "act"),
                    (1, 0, REM, "epi_restore", "act"),
                    (1, 1, REM, "epi_restore", "dve"),
                ])

            # ---------------- phase E: output projection + residual ----------------
            with tc.tile_pool(name="ps_e", bufs=4, space="PSUM") as PE_:
                for mc in range(4):
                    for tq in range(2):
                        yps = PE_.tile([P, 512], F32, tag="yps")
                        for kc in range(4):
                            nc.tensor.matmul(
                                yps[:], wo_sb[:, kc, ts(mc, 128)],
                                ohat[kc][:, ts(tq, 512)],
                                start=(kc == 0), stop=(kc == 3),
                            )
                        yt = TMP.tile([P, 512], F32, tag="yout")
                        nc.vector.tensor_add(yt[:], yps[:], xnb[mc][:, ts(tq, 512)])
                        if TRIVIAL_BO:
                            yo = yt
                        else:
                            yo = TMP.tile([P, 512], F32, tag="yout")
                            nc.vector.tensor_scalar_add(
                                yo[:], yt[:], bo_sb[:, mc : mc + 1]
                            )
                        for dq in range(2):
                            nc.sync.dma_start(
                                out_e[mc, :, ds(tq * 512 + dq * 256, 256)],
                                yo[:, ts(dq, 256)],
                            )

    nc.compile()
    return nc


def _host_inputs(x, gamma, beta, Wq, Wkv, Wout, bout):
    """Build the 8 per-core input maps."""
    x = np.asarray(x, np.float32)
    gamma = np.asarray(gamma, np.float32)
    beta = np.asarray(beta, np.float32)
    Wq = np.asarray(Wq, np.float32)
    Wkv = np.asarray(Wkv, np.float32)
    Wout = np.asarray(Wout, np.float32)
    bout = np.asarray(bout, np.float32)

    def swap_heads(W):
        # permute output cols j -> j xor 32 within each 64-block
        c = W.shape[1]
        return np.ascontiguousarray(
            W.reshape(D, c // 64, 2, 32)[:, :, ::-1, :].reshape(D, c)
        )

    def lhsT(W):
        # [D, M] -> [128, 4, M] chunk layout
        return np.ascontiguousarray(
            W.reshape(4, P, W.shape[1]).transpose(1, 0, 2).astype(bfdt)
        )

    Wk = Wkv[:, : KVH * DH]
    Wv = Wkv[:, KVH * DH :]
    wq = lhsT(Wq)
    wqs = lhsT(swap_heads(Wq))
    wk = lhsT(Wk)
    wks = lhsT(swap_heads(Wk))
    wv = lhsT(Wv)
    wo = lhsT(Wout)
    gam = np.ascontiguousarray(gamma.reshape(4, P).T)
    bet = np.ascontiguousarray(beta.reshape(4, P).T)
    bo = np.ascontiguousarray(bout.reshape(4, P).T)
    ones = np.ones((P, 1), np.float32)

    # rotary tables (per half)
    j = np.arange(DH)
    inv_freq = 1.0 / (10000.0 ** ((2.0 * (j % 32)) / DH))
    base = ((2.0 * (j % 32)) + 0.4 * DH) / (1.4 * DH)
    sign = np.where(j < 32, -1.0, 1.0)

    tables = []
    for half in range(2):
        pos = half * NLOC + np.arange(NLOC, dtype=np.float64)
        freqs = pos[None, :] * inv_freq[:, None]  # [64, NLOC]
        cos, sin = np.cos(freqs), np.sin(freqs)
        power = (pos - N // 2) / CONTEXT_LEN
        xsc = base[:, None] ** power[None, :]
        cq = np.tile((cos * xsc), (2, 1)).astype(bfdt)
        sq = np.tile((sign[:, None] * sin * xsc), (2, 1)).astype(bfdt)
        ck = np.tile((cos / xsc), (2, 1)).astype(bfdt)
        sk = np.tile((sign[:, None] * sin / xsc), (2, 1)).astype(bfdt)
        tables.append((cq, sq, ck, sk))

    in_maps = []
    for core in range(NCORES):
        b, half = core // 2, core % 2
        xc = np.ascontiguousarray(
            x[b].reshape(4, P, N)[:, :, half * NLOC : (half + 1) * NLOC]
        )
        cq, sq, ck, sk = tables[half]
        in_maps.append(
            {
                "x": xc, "wq": wq, "wqs": wqs, "wk": wk, "wks": wks,
                "wv": wv, "wo": wo, "cq": cq, "sq": sq, "ck": ck, "sk": sk,
                "gam": gam, "bet": bet, "bo": bo, "ones": ones,
                "onesb": ones.astype(bfdt),
            }
        )
    return in_maps


def kernel(x, gamma, beta, Wq, Wkv, Wout, bout):
    trace = os.environ.get("KERNEL_TRACE", "0") == "1"
    if trace:
        _enable_trace_hook()
    trivial_gb = bool(
        np.all(np.asarray(gamma) == 1.0) and np.all(np.asarray(beta) == 0.0)
    )
    trivial_bo = bool(np.all(np.asarray(bout) == 0.0))
    if "nc" not in _CACHE:
        _CACHE["nc"] = _build(trivial_gb, trivial_bo)
        _CACHE["trivial_gb"] = (trivial_gb, trivial_bo)
    assert _CACHE["trivial_gb"] == (trivial_gb, trivial_bo)
    nc = _CACHE["nc"]
    in_maps = _host_inputs(x, gamma, beta, Wq, Wkv, Wout, bout)
    res = run_bass_kernel_spmd(nc, in_maps, list(range(NCORES)), trace=trace)
    if trace and res.exec_time_ns is not None:
        print(f"HW exec time: {res.exec_time_ns} ns")
        _CACHE["exec_time_ns"] = res.exec_time_ns

    y = np.empty((B, D, N), np.float32)
    for core in range(NCORES):
        b, half = core // 2, core % 2
        y[b, :, half * NLOC : (half + 1) * NLOC] = res.results[core]["out"].reshape(
            D, NLOC
        )
    return y
